# revision 18
# baseline (speedup 1.0000x reference)
"""LSTM classifier kernel for Trainium2, data-parallel over batch on 8 cores.

v7: per-core batch slice of 16, fp16 matmuls, fp32 PSUM accumulation.
  - x-projection hoisted into an interleaved pre-GEMM over 128-token Mtiles
    (16 batch x 8 steps) with the full 128-wide stationary dim: 2x the PE
    efficiency of per-step M=16 x-matmuls.  One gate-chunk (512 cols) of the
    *next* Mtile is computed per step, between this step's transposes, so the
    PE never idles long enough for the HAM clock-gate to re-throttle.
    Bias is folded in via a ones-row matmul per chunk; the fp32 PSUM result
    is down-copied to an SBUF fp16 slab on the scalar engine.
  - Per step, a selector matmul (stationary [128,16] one-hot picking the
    step's 16 token rows, moving = slab chunk) deposits xW+bias into the
    col-tiled gates PSUM regions (i@0-15, f@32-47, o@64-79, g@96-111),
    doubling as the accumulation-group opener (start=True).
  - h-part matmuls ko-outer so the first 8 slots need only hT half a.
  - Pointwise in hidden-column halves as in v5 (fp16 DVE with PSUM-rebased
    operands; merged sigmoid over rows 0-80).

Self-contained: hardcodes shapes B=128, S=256, I=H=1024, C=1000, 8 cores.
"""

import numpy as np

import concourse.bass as bass
import concourse.mybir as mybir
import concourse.tile as tile
from concourse import bacc
from concourse import bass_utils
from concourse.masks import make_identity

F32 = mybir.dt.float32
F16 = mybir.dt.float16
AF = mybir.ActivationFunctionType
OP = mybir.AluOpType

B, S, I, H, C = 128, 256, 1024, 1024, 1000
NCORES = 8
BC = B // NCORES          # 16 batch rows per core
KO = H // 128             # 8 k-chunks
NG = H                    # per-gate width 1024
NH = 512                  # half width
GRP = 8                   # steps per Mtile (128 tokens)


def build_kernel(n_steps=S):
    assert n_steps % GRP == 0
    nmt = n_steps // GRP
    nc = bacc.Bacc("TRN2", target_bir_lowering=False, debug=False,
                   enable_asserts=False, num_devices=1)

    xt_d = nc.dram_tensor("xt", [I, n_steps * BC], F16, kind="ExternalInput")
    wxh_d = nc.dram_tensor("wxh", [I, 4, NG], F16, kind="ExternalInput")
    whh_d = nc.dram_tensor("whh", [I, 4, NG], F16, kind="ExternalInput")
    bias_d = nc.dram_tensor("bias", [1, 4, NG], F16, kind="ExternalInput")
    wfc_d = nc.dram_tensor("wfc", [I, 1024], F16, kind="ExternalInput")
    bfc_d = nc.dram_tensor("bfc", [1, 1024], F16, kind="ExternalInput")
    sel_d = nc.dram_tensor("selt", [128, GRP * BC], F16, kind="ExternalInput")
    out_d = nc.dram_tensor("out", [BC, C], F32, kind="ExternalOutput")

    with tile.TileContext(nc) as tc:
        with tc.tile_pool(name="const", bufs=1) as cpool, \
             tc.tile_pool(name="xp", bufs=2) as xpool, \
             tc.tile_pool(name="xw", bufs=2) as xwpool, \
             tc.tile_pool(name="tp", bufs=2) as tpool, \
             tc.tile_pool(name="ps", bufs=2, space="PSUM") as pspool, \
             tc.tile_pool(name="sc", bufs=1, space="PSUM") as scpool, \
             tc.tile_pool(name="px", bufs=1, space="PSUM") as pxpool, \
             tc.tile_pool(name="tps", bufs=1, space="PSUM") as tpspool:

            wxh = cpool.tile([128, KO, 4, NG], F16)
            whh = cpool.tile([128, KO, 4, NG], F16)
            nc.sync.dma_start(wxh[:, :, :, :],
                              wxh_d.ap().rearrange("(ko p) g n -> p ko g n", p=128))
            nc.sync.dma_start(whh[:, :, :, :],
                              whh_d.ap().rearrange("(ko p) g n -> p ko g n", p=128))
            bias_sb = cpool.tile([1, 4, NG], F16)
            nc.sync.dma_start(bias_sb[:, :, :], bias_d.ap()[:, :, :])
            wfc = cpool.tile([128, KO, 1024], F16)
            nc.sync.dma_start(wfc[:, :, :],
                              wfc_d.ap().rearrange("(ko p) n -> p ko n", p=128))
            bfc_sb = cpool.tile([1, 1024], F16)
            nc.sync.dma_start(bfc_sb[:, :], bfc_d.ap()[:, :])

            ones_sb = cpool.tile([1, BC], F16)
            nc.vector.memset(ones_sb[:, :], 1.0)
            ones128 = cpool.tile([1, 128], F16)
            nc.vector.memset(ones128[:, :], 1.0)
            ident64 = cpool.tile([80, BC], F16)
            make_identity(nc, ident64[64:80, :])
            # selector matrices: sel[:, t', :] one-hot rows 16t'..16t'+16
            # (built host-side: affine_select can't write at 16-aligned bases)
            sel = cpool.tile([128, GRP, BC], F16)
            nc.sync.dma_start(sel[:, :, :],
                              sel_d.ap().rearrange("p (t b) -> p t b", t=GRP))

            # persistent state, split per hidden half (a: ko 0-3, b: 4-7)
            hT = [[cpool.tile([128, KO // 2, BC], F16, name=f"hT{i}{hf}")
                   for hf in "ab"] for i in range(2)]
            for i in range(2):
                for hf in range(2):
                    nc.vector.memset(hT[i][hf][:, :, :], 0.0)
            # PSUM scratch per half: t2@0:16, c@32:48, tanh_g@96:112
            scr = [scpool.tile([128, NH], F32, name=f"scr{hf}") for hf in range(2)]
            nc.vector.memset(scr[0][32:48, :], 0.0)
            nc.vector.memset(scr[1][32:48, :], 0.0)

            xtr = xt_d.ap().rearrange("(ko p) t -> p ko t", p=128)

            def fetch_xmt(m):
                """DMA the 128-token stationary block for Mtile m."""
                xmt = xpool.tile([128, KO, 128], F16, tag="xmt", name="xmt")
                nc.sync.dma_start(xmt[:, :, :],
                                  xtr[:, :, m * 128:(m + 1) * 128])
                return xmt

            def mtile_chunk_p1(psx, xmt, g, nh):
                """Pre-GEMM first half (ko 0-3) of a 512-wide gate chunk."""
                nsl = slice(nh * NH, (nh + 1) * NH)
                for ko in range(4):
                    nc.tensor.matmul(psx[:, :], xmt[:, ko, :],
                                     wxh[:, ko, g, nsl],
                                     start=(ko == 0), stop=False,
                                     skip_group_check=True)

            def mtile_chunk_p2(psx, xmt, xw_sb, g, nh):
                """Second half (ko 4-7) + bias, then fp32 PSUM -> fp16 SBUF
                slab down-copy on the scalar engine."""
                nsl = slice(nh * NH, (nh + 1) * NH)
                for ko in range(4, KO):
                    nc.tensor.matmul(psx[:, :], xmt[:, ko, :],
                                     wxh[:, ko, g, nsl],
                                     start=False, stop=False,
                                     skip_group_check=True)
                nc.tensor.matmul(psx[:, :], ones128[:, :], bias_sb[:, g, nsl],
                                 start=False, stop=True, skip_group_check=True)
                nc.scalar.copy(xw_sb[:, g, nh, :], psx[:, :])

            def mtile_chunk(xmt, xw_sb, g, nh):
                psx = pxpool.tile([128, NH], F32, tag="psx", name="psx")
                mtile_chunk_p1(psx, xmt, g, nh)
                mtile_chunk_p2(psx, xmt, xw_sb, g, nh)

            def new_xw():
                return xwpool.tile([128, 4, 2, NH], F16, tag="xw", name="xw")

            # prologue: Mtile 0 fully (steps 0..7)
            xmt = fetch_xmt(0)
            xw_cur = new_xw()
            for g in range(4):
                for nh in range(2):
                    mtile_chunk(xmt, xw_cur, g, nh)
            if nmt > 1:
                xmt_nxt = fetch_xmt(1)
                xw_nxt = new_xw()

            for t in range(n_steps):
                tp_ = t % GRP
                ps = [pspool.tile([128, NH], F32, tag="gatesa", name="gatesa"),
                      pspool.tile([128, NH], F32, tag="gatesb", name="gatesb")]
                # deposit xW+bias via selector matmuls (opens the groups)
                for nh in range(2):
                    for g in range(4):
                        nc.tensor.matmul(
                            ps[nh][32 * g:32 * g + BC, :],
                            sel[:, tp_, :], xw_cur[:, g, nh, :],
                            tile_position=(0, 32 * g),
                            start=True, stop=False, skip_group_check=True)

                hTt = hT[t % 2]
                # h-part: ko outer so the first 8 slots only need hT half a
                for ko in range(KO):
                    src = hTt[0] if ko < 4 else hTt[1]
                    for nh in range(2):
                        nsl = slice(nh * NH, (nh + 1) * NH)
                        for g in range(4):
                            nc.tensor.matmul(
                                ps[nh][32 * g:32 * g + BC, :],
                                src[:, ko % 4, :], whh[:, ko, g, nsl],
                                tile_position=(0, 32 * g),
                                start=False, stop=(ko == KO - 1),
                                skip_group_check=True)

                # pointwise, per half
                acts = [tpool.tile([80, NH], F16, tag=f"acts{hf}",
                                   name=f"acts{hf}") for hf in range(2)]
                t1 = [tpool.tile([48, NH], F16, tag=f"t1{hf}",
                                 name=f"t1{hf}") for hf in range(2)]
                tcn = [tpool.tile([80, NH], F16, tag=f"tc{hf}",
                                  name=f"tc{hf}") for hf in range(2)]
                h16 = [tpool.tile([80, NH], F16, tag=f"h16{hf}",
                                  name=f"h16{hf}") for hf in range(2)]
                for hf in range(2):
                    nc.scalar.activation(acts[hf][0:80, :], ps[hf][0:80, :],
                                         AF.Sigmoid)
                    nc.scalar.activation(scr[hf][96:112, :], ps[hf][96:112, :],
                                         AF.Tanh)
                for hf in range(2):
                    nc.vector.tensor_tensor(t1[hf][32:48, :], acts[hf][32:48, :],
                                            scr[hf][32:48, :], OP.mult)
                    nc.vector.tensor_tensor(scr[hf][0:16, :], acts[hf][0:16, :],
                                            scr[hf][96:112, :], OP.mult)
                    nc.vector.tensor_tensor(scr[hf][32:48, :], t1[hf][32:48, :],
                                            scr[hf][0:16, :], OP.add)
                for hf in range(2):
                    nc.scalar.activation(tcn[hf][64:80, :], scr[hf][32:48, :],
                                         AF.Tanh)

                # pre-GEMM chunk of the NEXT Mtile (PE gap filler): one gate
                # chunk per step, split around the half-a transposes so the
                # PE FIFO has ready work while the pointwise chain runs
                m_nxt = t // GRP + 1
                do_pre = m_nxt < nmt
                if do_pre:
                    gq, nhq = divmod(tp_, 2)
                    psx = pxpool.tile([128, NH], F32, tag="psx", name="psx")
                    mtile_chunk_p1(psx, xmt_nxt, gq, nhq)

                hTn = hT[(t + 1) % 2]
                nc.vector.tensor_tensor(h16[0][64:80, :], acts[0][64:80, :],
                                        tcn[0][64:80, :], OP.mult)
                tps2 = tpspool.tile([128, 2, KO // 2, BC], F16, tag="tps",
                                    name="tps")
                for k in range(4):
                    nc.tensor.transpose(tps2[:, 0, k, :],
                                        h16[0][64:80, 128 * k:128 * (k + 1)],
                                        ident64[64:80, :])
                nc.vector.tensor_copy(hTn[0][:, :, :], tps2[:, 0, :, :])

                if do_pre:
                    mtile_chunk_p2(psx, xmt_nxt, xw_nxt, gq, nhq)

                nc.vector.tensor_tensor(h16[1][64:80, :], acts[1][64:80, :],
                                        tcn[1][64:80, :], OP.mult)
                for k in range(4):
                    nc.tensor.transpose(tps2[:, 1, k, :],
                                        h16[1][64:80, 128 * k:128 * (k + 1)],
                                        ident64[64:80, :])
                nc.vector.tensor_copy(hTn[1][:, :, :], tps2[:, 1, :, :])

                if do_pre and tp_ == GRP - 1:
                    # rotate buffers at group end
                    xw_cur = xw_nxt
                    if m_nxt + 1 < nmt:
                        xmt_nxt = fetch_xmt(m_nxt + 1)
                        xw_nxt = new_xw()

            # final FC: out = h_last @ WfcT + bfc
            hTl = hT[n_steps % 2]
            psf = [pspool.tile([128, NH], F32, tag="gatesa", name="gatesa"),
                   pspool.tile([128, NH], F32, tag="gatesb", name="gatesb")]
            for nh in range(2):
                nsl = slice(nh * NH, (nh + 1) * NH)
                for ko in range(KO):
                    src = hTl[0] if ko < 4 else hTl[1]
                    nc.tensor.matmul(psf[nh][0:BC, :], src[:, ko % 4, :],
                                     wfc[:, ko, nsl], start=(ko == 0), stop=False,
                                     skip_group_check=True)
                nc.tensor.matmul(psf[nh][0:BC, :], ones_sb[:, :],
                                 bfc_sb[:, nsl], start=False, stop=True,
                                 skip_group_check=True)
            out_sb = tpool.tile([BC, 1024], F32, tag="osb")
            nc.vector.tensor_copy(out_sb[:, 0:NH], psf[0][0:BC, :])
            nc.vector.tensor_copy(out_sb[:, NH:NG], psf[1][0:BC, :])
            nc.sync.dma_start(out_d.ap()[:, :], out_sb[:, 0:C])

    nc.compile()
    return nc


_NC_CACHE = {}


def _get_nc(n_steps=S):
    if n_steps not in _NC_CACHE:
        _NC_CACHE[n_steps] = build_kernel(n_steps)
    return _NC_CACHE[n_steps]


def _prep_weights(Wxh, bxh, Whh, bhh, Wfc, bfc):
    # gate order in reference along 4H: i, f, g(chat), o -> ours: i, f, o, g
    def arrange(WT):  # WT: [I, 4H]
        blocks = [WT[:, 0:H], WT[:, H:2 * H], WT[:, 3 * H:4 * H], WT[:, 2 * H:3 * H]]
        return np.ascontiguousarray(np.stack(blocks, axis=1)).astype(np.float16)

    wxh = arrange(Wxh.T.astype(np.float32))
    whh = arrange(Whh.T.astype(np.float32))
    b = (bxh + bhh).astype(np.float32)
    bias = np.stack([b[0:H], b[H:2 * H], b[3 * H:4 * H], b[2 * H:3 * H]],
                    axis=0)[None].astype(np.float16)
    wfc = np.zeros((I, 1024), np.float16)
    wfc[:, :C] = Wfc.T.astype(np.float16)
    selt = np.zeros((128, 8, 16), np.float16)
    for tp in range(8):
        for bb in range(16):
            selt[16 * tp + bb, tp, bb] = 1.0
    selt = selt.reshape(128, 128)
    bfc_p = np.zeros((1, 1024), np.float16)
    bfc_p[0, :C] = bfc.astype(np.float16)
    return wxh, whh, bias, wfc, bfc_p, selt


def kernel_run(x, Wxh, bxh, Whh, bhh, Wfc, bfc, n_steps=S, trace=False,
               tmpdir=None):
    x = np.asarray(x, np.float32)
    wxh, whh, bias, wfc, bfc_p, selt = _prep_weights(
        np.asarray(Wxh), np.asarray(bxh), np.asarray(Whh),
        np.asarray(bhh), np.asarray(Wfc), np.asarray(bfc))
    nc = _get_nc(n_steps)

    in_maps = []
    for core in range(NCORES):
        xc = x[core * BC:(core + 1) * BC, :n_steps]          # [16, S, I]
        xt = np.ascontiguousarray(
            xc.transpose(2, 1, 0).reshape(I, n_steps * BC)).astype(np.float16)
        in_maps.append(dict(xt=xt, wxh=wxh, whh=whh, bias=bias,
                            wfc=wfc, bfc=bfc_p, selt=selt))

    res = bass_utils.run_bass_kernel_spmd(
        nc, in_maps, core_ids=list(range(NCORES)), trace=trace,
        tmpdir=tmpdir)
    out = np.concatenate([r["out"] for r in res.results], axis=0)
    return out.astype(np.float32), res


def kernel(**inputs):
    out, _ = kernel_run(**inputs)
    return out


# revision 19
# speedup vs baseline: 1.2155x; 1.2155x over previous
"""LSTM classifier kernel for Trainium2, data-parallel over batch on 8 cores.

v7: per-core batch slice of 16, fp16 matmuls, fp32 PSUM accumulation.
  - x-projection hoisted into an interleaved pre-GEMM over 128-token Mtiles
    (16 batch x 8 steps) with the full 128-wide stationary dim: 2x the PE
    efficiency of per-step M=16 x-matmuls.  One gate-chunk (512 cols) of the
    *next* Mtile is computed per step, between this step's transposes, so the
    PE never idles long enough for the HAM clock-gate to re-throttle.
    Bias is folded in via a ones-row matmul per chunk; the fp32 PSUM result
    is down-copied to an SBUF fp16 slab on the scalar engine.
  - Per step, a selector matmul (stationary [128,16] one-hot picking the
    step's 16 token rows, moving = slab chunk) deposits xW+bias into the
    col-tiled gates PSUM regions (i@0-15, f@32-47, o@64-79, g@96-111),
    doubling as the accumulation-group opener (start=True).
  - h-part matmuls ko-outer so the first 8 slots need only hT half a.
  - Pointwise in hidden-column halves as in v5 (fp16 DVE with PSUM-rebased
    operands; merged sigmoid over rows 0-80).

Self-contained: hardcodes shapes B=128, S=256, I=H=1024, C=1000, 8 cores.
"""

import numpy as np

import concourse.bass as bass
import concourse.mybir as mybir
import concourse.tile as tile
from concourse import bacc
from concourse import bass_utils
from concourse.masks import make_identity

F32 = mybir.dt.float32
F16 = mybir.dt.float16
AF = mybir.ActivationFunctionType
OP = mybir.AluOpType

B, S, I, H, C = 128, 256, 1024, 1024, 1000
NCORES = 8
BC = B // NCORES          # 16 batch rows per core
KO = H // 128             # 8 k-chunks
NG = H                    # per-gate width 1024
NH = 512                  # half width
GRP = 8                   # steps per Mtile (128 tokens)


def build_kernel(n_steps=S):
    assert n_steps % GRP == 0
    nmt = n_steps // GRP
    nc = bacc.Bacc("TRN2", target_bir_lowering=False, debug=False,
                   enable_asserts=False, num_devices=1)

    xt_d = nc.dram_tensor("xt", [I, n_steps * BC], F16, kind="ExternalInput")
    wxh_d = nc.dram_tensor("wxh", [I, 4, NG], F16, kind="ExternalInput")
    whh_d = nc.dram_tensor("whh", [I, 4, NG], F16, kind="ExternalInput")
    bias_d = nc.dram_tensor("bias", [1, 4, NG], F16, kind="ExternalInput")
    wfc_d = nc.dram_tensor("wfc", [I, 1024], F16, kind="ExternalInput")
    bfc_d = nc.dram_tensor("bfc", [1, 1024], F16, kind="ExternalInput")
    sel_d = nc.dram_tensor("selt", [128, GRP * BC], F16, kind="ExternalInput")
    out_d = nc.dram_tensor("out", [BC, C], F32, kind="ExternalOutput")

    with tile.TileContext(nc) as tc:
        with tc.tile_pool(name="const", bufs=1) as cpool, \
             tc.tile_pool(name="xp", bufs=2) as xpool, \
             tc.tile_pool(name="xw", bufs=2) as xwpool, \
             tc.tile_pool(name="tp", bufs=2) as tpool, \
             tc.tile_pool(name="ps", bufs=2, space="PSUM") as pspool, \
             tc.tile_pool(name="sc", bufs=1, space="PSUM") as scpool, \
             tc.tile_pool(name="px", bufs=1, space="PSUM") as pxpool, \
             tc.tile_pool(name="tps", bufs=1, space="PSUM") as tpspool:

            wxh = cpool.tile([128, KO, 4, NG], F16)
            whh = cpool.tile([128, KO, 4, NG], F16)
            nc.sync.dma_start(wxh[:, :, :, :],
                              wxh_d.ap().rearrange("(ko p) g n -> p ko g n", p=128))
            nc.sync.dma_start(whh[:, :, :, :],
                              whh_d.ap().rearrange("(ko p) g n -> p ko g n", p=128))
            bias_sb = cpool.tile([1, 4, NG], F16)
            nc.sync.dma_start(bias_sb[:, :, :], bias_d.ap()[:, :, :])
            wfc = cpool.tile([128, KO, 1024], F16)
            nc.sync.dma_start(wfc[:, :, :],
                              wfc_d.ap().rearrange("(ko p) n -> p ko n", p=128))
            bfc_sb = cpool.tile([1, 1024], F16)
            nc.sync.dma_start(bfc_sb[:, :], bfc_d.ap()[:, :])

            ones_sb = cpool.tile([1, BC], F16)
            nc.vector.memset(ones_sb[:, :], 1.0)
            ones128 = cpool.tile([1, 128], F16)
            nc.vector.memset(ones128[:, :], 1.0)
            ident64 = cpool.tile([80, BC], F16)
            make_identity(nc, ident64[64:80, :])
            # selector matrices: sel[:, t', :] one-hot rows 16t'..16t'+16
            # (built host-side: affine_select can't write at 16-aligned bases)
            sel = cpool.tile([128, GRP, BC], F16)
            nc.sync.dma_start(sel[:, :, :],
                              sel_d.ap().rearrange("p (t b) -> p t b", t=GRP))

            # persistent state, split per hidden half (a: ko 0-3, b: 4-7)
            hT = [[cpool.tile([128, KO // 2, BC], F16, name=f"hT{i}{hf}")
                   for hf in "ab"] for i in range(2)]
            for i in range(2):
                for hf in range(2):
                    nc.vector.memset(hT[i][hf][:, :, :], 0.0)
            # PSUM scratch per half: t2@0:16, c@32:48, tanh_g@96:112
            scr = [scpool.tile([128, NH], F32, name=f"scr{hf}") for hf in range(2)]
            nc.vector.memset(scr[0][32:48, :], 0.0)
            nc.vector.memset(scr[1][32:48, :], 0.0)

            xtr = xt_d.ap().rearrange("(ko p) t -> p ko t", p=128)

            def fetch_xmt(m):
                """DMA the 128-token stationary block for Mtile m."""
                xmt = xpool.tile([128, KO, 128], F16, tag="xmt", name="xmt")
                nc.sync.dma_start(xmt[:, :, :],
                                  xtr[:, :, m * 128:(m + 1) * 128])
                return xmt

            def mtile_chunk_p1(psx, xmt, g, nh):
                """Pre-GEMM first half (ko 0-3) of a 512-wide gate chunk."""
                nsl = slice(nh * NH, (nh + 1) * NH)
                for ko in range(4):
                    nc.tensor.matmul(psx[:, :], xmt[:, ko, :],
                                     wxh[:, ko, g, nsl],
                                     start=(ko == 0), stop=False,
                                     skip_group_check=True)

            def mtile_chunk_p2(psx, xmt, xw_sb, g, nh):
                """Second half (ko 4-7) + bias, then fp32 PSUM -> fp16 SBUF
                slab down-copy on the scalar engine."""
                nsl = slice(nh * NH, (nh + 1) * NH)
                for ko in range(4, KO):
                    nc.tensor.matmul(psx[:, :], xmt[:, ko, :],
                                     wxh[:, ko, g, nsl],
                                     start=False, stop=False,
                                     skip_group_check=True)
                nc.tensor.matmul(psx[:, :], ones128[:, :], bias_sb[:, g, nsl],
                                 start=False, stop=True, skip_group_check=True)

            def mtile_chunk(xmt, xw_sb, g, nh):
                psx = pxpool.tile([128, NH], F32, tag="psx", name="psx")
                mtile_chunk_p1(psx, xmt, g, nh)
                mtile_chunk_p2(psx, xmt, xw_sb, g, nh)
                nc.scalar.copy(xw_sb[:, g, nh, :], psx[:, :])

            def new_xw():
                return xwpool.tile([128, 4, 2, NH], F16, tag="xw", name="xw")

            # prologue: Mtile 0 fully (steps 0..7)
            xmt = fetch_xmt(0)
            xw_cur = new_xw()
            for g in range(4):
                for nh in range(2):
                    mtile_chunk(xmt, xw_cur, g, nh)
            if nmt > 1:
                xmt_nxt = fetch_xmt(1)
                xw_nxt = new_xw()

            pending = []
            for t in range(n_steps):
                tp_ = t % GRP
                ps = [pspool.tile([128, NH], F32, tag="gatesa", name="gatesa"),
                      pspool.tile([128, NH], F32, tag="gatesb", name="gatesb")]
                # deposit xW+bias via selector matmuls (opens the groups)
                for nh in range(2):
                    for g in range(4):
                        nc.tensor.matmul(
                            ps[nh][32 * g:32 * g + BC, :],
                            sel[:, tp_, :], xw_cur[:, g, nh, :],
                            tile_position=(0, 32 * g),
                            start=True, stop=False, skip_group_check=True)

                hTt = hT[t % 2]
                # h-part: ko outer so the first 8 slots only need hT half a
                for ko in range(KO):
                    src = hTt[0] if ko < 4 else hTt[1]
                    for nh in range(2):
                        nsl = slice(nh * NH, (nh + 1) * NH)
                        for g in range(4):
                            nc.tensor.matmul(
                                ps[nh][32 * g:32 * g + BC, :],
                                src[:, ko % 4, :], whh[:, ko, g, nsl],
                                tile_position=(0, 32 * g),
                                start=False, stop=(ko == KO - 1),
                                skip_group_check=True)

                # pointwise, per half
                acts = [tpool.tile([80, NH], F16, tag=f"acts{hf}",
                                   name=f"acts{hf}") for hf in range(2)]
                t1 = [tpool.tile([48, NH], F16, tag=f"t1{hf}",
                                 name=f"t1{hf}") for hf in range(2)]
                tcn = [tpool.tile([80, NH], F16, tag=f"tc{hf}",
                                  name=f"tc{hf}") for hf in range(2)]
                h16 = [tpool.tile([80, NH], F16, tag=f"h16{hf}",
                                  name=f"h16{hf}") for hf in range(2)]
                for hf in range(2):
                    nc.scalar.activation(acts[hf][0:80, :], ps[hf][0:80, :],
                                         AF.Sigmoid)
                    nc.scalar.activation(scr[hf][96:112, :], ps[hf][96:112, :],
                                         AF.Tanh)
                # deferred slab down-copies from the previous step's chunk:
                # this ACT slot runs while ACT would otherwise idle for c,
                # and unblocks the psx WAR for this step's filler matmuls
                for (ppsx, pxw, pg, pnh) in pending:
                    nc.scalar.copy(pxw[:, pg, pnh, :], ppsx[:, :])
                pending = []
                for hf in range(2):
                    nc.vector.tensor_tensor(t1[hf][32:48, :], acts[hf][32:48, :],
                                            scr[hf][32:48, :], OP.mult)
                    nc.vector.tensor_tensor(scr[hf][0:16, :], acts[hf][0:16, :],
                                            scr[hf][96:112, :], OP.mult)
                    nc.vector.tensor_tensor(scr[hf][32:48, :], t1[hf][32:48, :],
                                            scr[hf][0:16, :], OP.add)
                for hf in range(2):
                    nc.scalar.activation(tcn[hf][64:80, :], scr[hf][32:48, :],
                                         AF.Tanh)

                # pre-GEMM chunk of the NEXT Mtile (PE gap filler): one gate
                # chunk per step, split around the half-a transposes so the
                # PE FIFO has ready work while the pointwise chain runs
                m_nxt = t // GRP + 1
                do_pre = m_nxt < nmt
                if do_pre:
                    gq, nhq = divmod(tp_, 2)
                    psx = pxpool.tile([128, NH], F32, tag="psx", name="psx")
                    mtile_chunk_p1(psx, xmt_nxt, gq, nhq)

                hTn = hT[(t + 1) % 2]
                nc.vector.tensor_tensor(h16[0][64:80, :], acts[0][64:80, :],
                                        tcn[0][64:80, :], OP.mult)
                tps2 = tpspool.tile([128, 2, KO // 2, BC], F16, tag="tps",
                                    name="tps")
                for k in range(4):
                    nc.tensor.transpose(tps2[:, 0, k, :],
                                        h16[0][64:80, 128 * k:128 * (k + 1)],
                                        ident64[64:80, :])
                nc.vector.tensor_copy(hTn[0][:, :, :], tps2[:, 0, :, :])

                if do_pre:
                    mtile_chunk_p2(psx, xmt_nxt, xw_nxt, gq, nhq)
                    if tp_ == GRP - 1:
                        # group boundary: next step's selectors need the slab
                        nc.scalar.copy(xw_nxt[:, gq, nhq, :], psx[:, :])
                    else:
                        pending.append((psx, xw_nxt, gq, nhq))

                nc.vector.tensor_tensor(h16[1][64:80, :], acts[1][64:80, :],
                                        tcn[1][64:80, :], OP.mult)
                for k in range(4):
                    nc.tensor.transpose(tps2[:, 1, k, :],
                                        h16[1][64:80, 128 * k:128 * (k + 1)],
                                        ident64[64:80, :])
                nc.vector.tensor_copy(hTn[1][:, :, :], tps2[:, 1, :, :])

                if do_pre and tp_ == GRP - 1:
                    # rotate buffers at group end
                    xw_cur = xw_nxt
                    if m_nxt + 1 < nmt:
                        xmt_nxt = fetch_xmt(m_nxt + 1)
                        xw_nxt = new_xw()

            # final FC: out = h_last @ WfcT + bfc
            hTl = hT[n_steps % 2]
            psf = [pspool.tile([128, NH], F32, tag="gatesa", name="gatesa"),
                   pspool.tile([128, NH], F32, tag="gatesb", name="gatesb")]
            for nh in range(2):
                nsl = slice(nh * NH, (nh + 1) * NH)
                for ko in range(KO):
                    src = hTl[0] if ko < 4 else hTl[1]
                    nc.tensor.matmul(psf[nh][0:BC, :], src[:, ko % 4, :],
                                     wfc[:, ko, nsl], start=(ko == 0), stop=False,
                                     skip_group_check=True)
                nc.tensor.matmul(psf[nh][0:BC, :], ones_sb[:, :],
                                 bfc_sb[:, nsl], start=False, stop=True,
                                 skip_group_check=True)
            out_sb = tpool.tile([BC, 1024], F32, tag="osb")
            nc.vector.tensor_copy(out_sb[:, 0:NH], psf[0][0:BC, :])
            nc.vector.tensor_copy(out_sb[:, NH:NG], psf[1][0:BC, :])
            nc.sync.dma_start(out_d.ap()[:, :], out_sb[:, 0:C])

    nc.compile()
    return nc


_NC_CACHE = {}


def _get_nc(n_steps=S):
    if n_steps not in _NC_CACHE:
        _NC_CACHE[n_steps] = build_kernel(n_steps)
    return _NC_CACHE[n_steps]


def _prep_weights(Wxh, bxh, Whh, bhh, Wfc, bfc):
    # gate order in reference along 4H: i, f, g(chat), o -> ours: i, f, o, g
    def arrange(WT):  # WT: [I, 4H]
        blocks = [WT[:, 0:H], WT[:, H:2 * H], WT[:, 3 * H:4 * H], WT[:, 2 * H:3 * H]]
        return np.ascontiguousarray(np.stack(blocks, axis=1)).astype(np.float16)

    wxh = arrange(Wxh.T.astype(np.float32))
    whh = arrange(Whh.T.astype(np.float32))
    b = (bxh + bhh).astype(np.float32)
    bias = np.stack([b[0:H], b[H:2 * H], b[3 * H:4 * H], b[2 * H:3 * H]],
                    axis=0)[None].astype(np.float16)
    wfc = np.zeros((I, 1024), np.float16)
    wfc[:, :C] = Wfc.T.astype(np.float16)
    selt = np.zeros((128, 8, 16), np.float16)
    for tp in range(8):
        for bb in range(16):
            selt[16 * tp + bb, tp, bb] = 1.0
    selt = selt.reshape(128, 128)
    bfc_p = np.zeros((1, 1024), np.float16)
    bfc_p[0, :C] = bfc.astype(np.float16)
    return wxh, whh, bias, wfc, bfc_p, selt


def kernel_run(x, Wxh, bxh, Whh, bhh, Wfc, bfc, n_steps=S, trace=False,
               tmpdir=None):
    x = np.asarray(x, np.float32)
    wxh, whh, bias, wfc, bfc_p, selt = _prep_weights(
        np.asarray(Wxh), np.asarray(bxh), np.asarray(Whh),
        np.asarray(bhh), np.asarray(Wfc), np.asarray(bfc))
    nc = _get_nc(n_steps)

    in_maps = []
    for core in range(NCORES):
        xc = x[core * BC:(core + 1) * BC, :n_steps]          # [16, S, I]
        xt = np.ascontiguousarray(
            xc.transpose(2, 1, 0).reshape(I, n_steps * BC)).astype(np.float16)
        in_maps.append(dict(xt=xt, wxh=wxh, whh=whh, bias=bias,
                            wfc=wfc, bfc=bfc_p, selt=selt))

    res = bass_utils.run_bass_kernel_spmd(
        nc, in_maps, core_ids=list(range(NCORES)), trace=trace,
        tmpdir=tmpdir)
    out = np.concatenate([r["out"] for r in res.results], axis=0)
    return out.astype(np.float32), res


def kernel(**inputs):
    out, _ = kernel_run(**inputs)
    return out
